# revision 15
# baseline (speedup 1.0000x reference)
"""Multi-head self-attention (B=2, S=2048, D=1024, H=16) on 8 Trainium2 NeuronCores.

Sharding: batch x head-group. Core c = b*4 + g handles batch b and heads 4g..4g+3
(Megatron-style TP: Wq/Wk/Wv column-sharded, Wo row-sharded; partial outputs
summed on the host).

v3 design (bf16 compute, fp32 PSUM accumulation), T-layout (sequence on the
free dim everywhere):
  QT/KT = (w.T @ xt) [256, 2048]      d' on partitions
  V     = (xt.T @ wv) [2048, 260]     natural layout + ones column per head,
                                      K=64 row-tiled pair chains (hides LDW)
  scoresT[k, q] = KT_h.T @ QT_h       per head, K=64 row pairs (2 heads
                                      concurrent in rows 0-63 / 64-127)
  expT = exp(scoresT / 8)             bf16, scalar engine (no max subtraction:
                                      |scores| <~ 2)
  ctxT_aug[d'+1, q] = [V_h | 1].T @ expT   K=128 chains; row 64 = denominator
  ctxT = ctxT_aug[0:64] * (1/denom)   recip_approx_fast + gpsimd
                                      partition_broadcast + DVE mult
  outT_partial = wo.T @ ctxT          bf16 out, host sums partials

Scheduling: the scalar-engine exp stream (~143us) is the critical path, so
the emission order keeps it fed from ~18us on:
  - xs is DMA'd per-ko (4KB lines) and the lead-in runs KT-m0 as 4 parallel
    ko-outer PSUM chains + QT-m0-n0, so the first scores fire as soon as the
    last xs chunk lands.
  - loop is head-pair-OUTER: hp=0 blocks only need the m=0 projections; all
    m=1 projections, V pair-chains, and output projections are spread as PE
    fill work into the exp-bound kc loops of later blocks.
"""
import sys

sys.path.insert(0, "/opt/trn_rl_repo")

import numpy as np
import ml_dtypes

import concourse.bass as bass
import concourse.tile as tile
from concourse import bacc, library_config, mybir
from concourse.bass_utils import run_bass_kernel_spmd

F32 = mybir.dt.float32
BF16 = mybir.dt.bfloat16

S = 2048          # sequence length per batch
D = 1024          # embedding dim
HG = 4            # heads per core
HD = 64           # head dim
GC = HG * HD      # group cols = 256
P = 128
NQ = 4            # q chunks of 512
QW = 512          # q chunk width
NKC = 16          # key-position chunks of 128
KO = 8            # contraction chunks of 128 over D
VW = HD + 1       # V columns per head incl. ones column

_NC_CACHE = {}
DEBUG_DUMPS = False


def _build():
    if "nc" in _NC_CACHE:
        return _NC_CACHE["nc"]
    nc = bacc.Bacc(trn_type="TRN2", target_bir_lowering=False, debug=False)
    xt_d = nc.dram_tensor("xt", [D, S], BF16, kind="ExternalInput")
    wq_d = nc.dram_tensor("wq", [D, GC], BF16, kind="ExternalInput")
    wk_d = nc.dram_tensor("wk", [D, GC], BF16, kind="ExternalInput")
    wv_d = nc.dram_tensor("wv", [D, GC], BF16, kind="ExternalInput")
    wo_d = nc.dram_tensor("wo", [GC, D], BF16, kind="ExternalInput")
    out_d = nc.dram_tensor("out_t", [D, S], BF16, kind="ExternalOutput")
    with tile.TileContext(nc) as tc:
        _emit(nc, tc, xt_d, wq_d, wk_d, wv_d, wo_d, out_d)
    nc.compile()
    _NC_CACHE["nc"] = nc
    return nc


def _emit(nc, tc, xt_d, wq_d, wk_d, wv_d, wo_d, out_d):
    with tc.tile_pool(name="big", bufs=1) as big, \
         tc.tile_pool(name="expp", bufs=4) as expp, \
         tc.tile_pool(name="norm", bufs=2) as norm, \
         tc.tile_pool(name="evac", bufs=2) as evac, \
         tc.tile_pool(name="outp", bufs=3) as outp, \
         tc.tile_pool(name="ps_sc", bufs=2, space="PSUM") as ps_sc, \
         tc.tile_pool(name="ps_ctx", bufs=1, space="PSUM") as ps_ctx, \
         tc.tile_pool(name="ps_o", bufs=2, space="PSUM") as ps_o:
        # ---- persistent SBUF tensors (~80KB/partition, bf16) ----
        xs = big.tile([P, KO, S], BF16)         # x.T, [d_in(128) x ko x s]
        wqs = big.tile([P, KO, GC], BF16)
        wks = big.tile([P, KO, GC], BF16)
        wvs = big.tile([P, KO, GC], BF16)
        wos = big.tile([P, 2, D], BF16)         # [d'(128) x chunk x e]
        qt = big.tile([P, 2, S], BF16)          # head h at parts (h%2)*64, chunk h//2
        kt = big.tile([P, 2, S], BF16)
        va = big.tile([P, NKC, HG * VW], BF16)  # V natural + ones col per head
        ct = big.tile([P, 2, S], BF16)          # ctxT, same head layout as qt

        # partition_broadcast runs on the Q7 cores and needs the attn ucode
        # library resident (CoreSim doesn't care, hardware does).
        nc.gpsimd.load_library(library_config.attn)

        # DMA issue spread across engine queues; xs per-ko so the ko-outer
        # lead-in chains start after the first chunk, not the whole tensor.
        nc.scalar.dma_start(wks[:], wk_d.rearrange("(ko p) m -> p ko m", p=P))
        nc.scalar.dma_start(wqs[:], wq_d.rearrange("(ko p) m -> p ko m", p=P))
        xt_r = xt_d.rearrange("(ko p) s -> p ko s", p=P)
        for ko in range(KO):
            nc.sync.dma_start(xs[:, ko, :], xt_r[:, ko, :])
        nc.scalar.dma_start(wvs[:], wv_d.rearrange("(ko p) m -> p ko m", p=P))
        nc.scalar.dma_start(wos[:], wo_d.rearrange("(c p) e -> p c e", p=P))

        # ones columns of V_aug (col HD of each VW-wide head block)
        va_h = va[:].rearrange("p s (h c) -> p s h c", c=VW)
        for h in range(HG):
            nc.vector.memset(
                va_h[:, :, h, HD:HD + 1].bitcast(mybir.dt.uint16), 0x3F80)

        def proj_chain(w_sb, m, n, dst):
            """dst[:, m, n*QW:] = sum_ko w_sb[:,ko,m*128:+128].T @ xs[:,ko,nq]"""
            pp = ps_o.tile([P, QW], F32, tag="po", name=f"pj_{m}_{n}")
            for ko in range(KO):
                nc.tensor.matmul(pp[:], w_sb[:, ko, m * P:(m + 1) * P],
                                 xs[:, ko, n * QW:(n + 1) * QW],
                                 start=(ko == 0), stop=(ko == KO - 1))
            nc.vector.tensor_copy(dst[:, m, n * QW:(n + 1) * QW], pp[:])

        def v_chain(kc):
            """va[:, kc, heads] = xs[:, :, kc-chunk].T @ wv as K=64 row pairs
            into two banks (hides LDWEIGHTS), combined by the DVE add."""
            pv = [ps_o.tile([P, QW], F32, tag="po", name=f"pv{l}_{kc}")
                  for l in range(2)]
            for ko in range(KO):
                for l in range(2):
                    lo = l * 64
                    nc.tensor.matmul(pv[l][:, 0:GC],
                                     xs[lo:lo + 64, ko, kc * P:(kc + 1) * P],
                                     wvs[lo:lo + 64, ko, :],
                                     start=(ko == 0), stop=(ko == KO - 1))
            # DVE can read only one PSUM operand per instruction
            tv = evac.tile([P, GC], F32, tag="ev")
            nc.vector.tensor_copy(tv[:], pv[1][:, 0:GC])
            nc.vector.tensor_tensor(
                va_h[:, kc, :, 0:HD],
                pv[0][:, 0:GC].rearrange("p (h c) -> p h c", c=HD),
                tv[:].rearrange("p (h c) -> p h c", c=HD),
                mybir.AluOpType.add)

        def po_chain(mo, n):
            """out_t[mo*128:+128, nq] = sum_c wos[:,c,mo*128:+128].T @ ct[:,c,nq]"""
            pp = ps_o.tile([P, QW], F32, tag="po", name=f"po_{mo}_{n}")
            for c in range(2):
                nc.tensor.matmul(pp[:], wos[:, c, mo * P:(mo + 1) * P],
                                 ct[:, c, n * QW:(n + 1) * QW],
                                 start=(c == 0), stop=(c == 1))
            ot = outp.tile([P, QW], BF16, tag="ot")
            nc.vector.tensor_copy(ot[:], pp[:])
            nc.sync.dma_start(
                out_d[mo * P:(mo + 1) * P, n * QW:(n + 1) * QW], ot[:])

        # ---- lead-in: ko-outer KT-m0 (4 parallel chains) + QT-m0-n0, so the
        # first matmul fires when the first xs chunk lands.
        ktp = [ps_sc.tile([P, 2, QW], F32, tag="psc", name=f"lead{i}")
               for i in range(2)]
        qtp = ps_o.tile([P, QW], F32, tag="po", name="lead_q")
        for ko in range(KO):
            for n in range(NQ):
                nc.tensor.matmul(ktp[n // 2][:, n % 2, :],
                                 wks[:, ko, 0:P],
                                 xs[:, ko, n * QW:(n + 1) * QW],
                                 start=(ko == 0), stop=(ko == KO - 1))
            nc.tensor.matmul(qtp[:], wqs[:, ko, 0:P], xs[:, ko, 0:QW],
                             start=(ko == 0), stop=(ko == KO - 1))
        for n in range(NQ):
            nc.vector.tensor_copy(kt[:, 0, n * QW:(n + 1) * QW],
                                  ktp[n // 2][:, n % 2, :])
        nc.vector.tensor_copy(qt[:, 0, 0:QW], qtp[:])

        # ---- fill work spread into the exp-bound kc loops ----
        # (hp, n) -> list of (kc, thunk)
        fills = {
            (0, 0): [(kc, lambda kc=kc: v_chain(kc)) for kc in range(NKC)]
                    + [(12, lambda: proj_chain(wqs, 0, 1, qt))],
            (0, 1): [(3, lambda: proj_chain(wqs, 0, 2, qt)),
                     (6, lambda: proj_chain(wks, 1, 0, kt)),
                     (10, lambda: proj_chain(wks, 1, 1, kt))],
            (0, 2): [(3, lambda: proj_chain(wqs, 0, 3, qt)),
                     (7, lambda: proj_chain(wks, 1, 2, kt)),
                     (11, lambda: proj_chain(wks, 1, 3, kt))],
            (0, 3): [(4, lambda: proj_chain(wqs, 1, 0, qt))],
            (1, 0): [(4, lambda: proj_chain(wqs, 1, 1, qt))],
            (1, 1): [(2, lambda: proj_chain(wqs, 1, 2, qt))]
                    + [(8 + mo, lambda mo=mo: po_chain(mo, 0)) for mo in range(KO)],
            (1, 2): [(2, lambda: proj_chain(wqs, 1, 3, qt))]
                    + [(8 + mo, lambda mo=mo: po_chain(mo, 1)) for mo in range(KO)],
            (1, 3): [(8 + mo, lambda mo=mo: po_chain(mo, 2)) for mo in range(KO)],
        }

        # ---- main loop: head-pair hp OUTER, q-block n inner ----
        for hp in range(2):
            for n in range(NQ):
                fill = sorted(fills.get((hp, n), []), key=lambda t: t[0])
                fi = 0
                cps = [ps_ctx.tile([VW, QW], F32, tag=f"pc{e}",
                                   name=f"pc{e}_{hp}_{n}") for e in range(2)]
                for kc in range(NKC):
                    sp = ps_sc.tile([P, 2, QW], F32, tag="psc")
                    for e in range(2):   # head 2hp+e in rows e*64..e*64+63
                        lo = e * HD
                        nc.tensor.matmul(
                            sp[:, e, :],
                            kt[lo:lo + HD, hp, kc * P:(kc + 1) * P],
                            qt[lo:lo + HD, hp, n * QW:(n + 1) * QW],
                            start=True, stop=True)
                    while fi < len(fill) and fill[fi][0] <= kc:
                        fill[fi][1]()
                        fi += 1
                    ex = expp.tile([P, 2, QW], BF16, tag="pex")
                    nc.scalar.activation(
                        ex[:].rearrange("p a b -> p (a b)"),
                        sp[:].rearrange("p a b -> p (a b)"),
                        mybir.ActivationFunctionType.Exp,
                        scale=0.125)
                    for e in range(2):
                        h = 2 * hp + e
                        nc.tensor.matmul(
                            cps[e][:],
                            va[:, kc, h * VW:(h + 1) * VW],
                            ex[:, e, :],
                            start=(kc == 0), stop=(kc == NKC - 1))
                while fi < len(fill):
                    fill[fi][1]()
                    fi += 1
                # normalize: ctxT = cps[0:64] * (1 / cps[64])
                for e in range(2):
                    # custom-DVE ops drop the input base-partition on HW, so
                    # stage the denominator row to SBUF partition 0 first.
                    dsb = norm.tile([1, QW], F32, tag="nd")
                    nc.vector.tensor_copy(dsb[:], cps[e][HD:VW, :])
                    r = norm.tile([1, QW], F32, tag="nr")
                    nc.vector.reciprocal_approx_fast(r[:], dsb[:])
                    bc = norm.tile([HD, QW], F32, tag="nb")
                    nc.gpsimd.partition_broadcast(bc[:], r[:])
                    nc.vector.tensor_tensor(
                        ct[e * HD:(e + 1) * HD, hp, n * QW:(n + 1) * QW],
                        cps[e][0:HD, :], bc[:], mybir.AluOpType.mult)
        # ---- tail: output projection of the last q-block ----
        for mo in range(KO):
            po_chain(mo, NQ - 1)


def _in_maps(x, wq_f, wk_f, wv_f, wo_f):
    bf = ml_dtypes.bfloat16
    maps = []
    for core in range(8):
        b, g = core // 4, core % 4
        cols = slice(g * GC, (g + 1) * GC)
        maps.append({
            "xt": np.ascontiguousarray(x[b].T).astype(bf),
            "wq": np.ascontiguousarray(wq_f[:, cols]).astype(bf),
            "wk": np.ascontiguousarray(wk_f[:, cols]).astype(bf),
            "wv": np.ascontiguousarray(wv_f[:, cols]).astype(bf),
            "wo": np.ascontiguousarray(wo_f[cols, :]).astype(bf),
        })
    return maps


def _prep(x, Wq, Wk, Wv, Wo, q_scale, k_scale, v_scale, o_scale):
    x = np.asarray(x, dtype=np.float32)
    wq_f = (np.asarray(Wq).T * np.asarray(q_scale).reshape(1, -1)).astype(np.float32)
    wk_f = (np.asarray(Wk).T * np.asarray(k_scale).reshape(1, -1)).astype(np.float32)
    wv_f = (np.asarray(Wv).T * np.asarray(v_scale).reshape(1, -1)).astype(np.float32)
    wo_f = (np.asarray(Wo).T * np.asarray(o_scale).reshape(1, -1)).astype(np.float32)
    return x, wq_f, wk_f, wv_f, wo_f


def _gather(res, B):
    out = np.zeros((B, S, D), dtype=np.float32)
    for core in range(8):
        out[core // 4] += res.results[core]["out_t"].astype(np.float32).T
    return out


def run_traced(x, Wq, Wk, Wv, Wo, q_scale, k_scale, v_scale, o_scale):
    """Like kernel() but with NTFF tracing; returns (out, exec_time_ns, trace_path)."""
    x, wq_f, wk_f, wv_f, wo_f = _prep(x, Wq, Wk, Wv, Wo,
                                      q_scale, k_scale, v_scale, o_scale)
    nc = _build()
    res = run_bass_kernel_spmd(nc, _in_maps(x, wq_f, wk_f, wv_f, wo_f),
                               core_ids=list(range(8)), trace=True)
    out = _gather(res, x.shape[0])
    trace_path = None
    if res.instructions_and_trace is not None:
        trace_path = res.instructions_and_trace[1]
    return out, res.exec_time_ns, trace_path


def kernel(x, Wq, Wk, Wv, Wo, q_scale, k_scale, v_scale, o_scale):
    B = x.shape[0]
    x, wq_f, wk_f, wv_f, wo_f = _prep(x, Wq, Wk, Wv, Wo,
                                      q_scale, k_scale, v_scale, o_scale)
    nc = _build()
    res = run_bass_kernel_spmd(nc, _in_maps(x, wq_f, wk_f, wv_f, wo_f),
                               core_ids=list(range(8)))
    return _gather(res, B)
